# revision 8
# baseline (speedup 1.0000x reference)
"""Multi-head attention (B=4,T=2048,C=2048,H=16, causal, full-dim RoPE) on 8 TRN2 cores.

Strategy: tensor-parallel over heads (2 heads/core, Megatron-style).
Per core: QKV projection (bf16 matmuls, f32 accum) -> RoPE on q,k (DVE)
-> PE-transpose q,k to [ch, tok] -> causal attention per (batch, head)
with exp on ACT and denominator via ones-column in v -> output proj
partial sums -> HBM. Host sums the 8 partial proj outputs + bias.

Raw Bass (explicit per-engine streams + semaphores); Tile's inline waits
don't compile on this toolchain.
"""
import numpy as np
import ml_dtypes

import concourse.bass as bass
import concourse.mybir as mybir
from concourse.masks import make_identity
from concourse.bass_utils import run_bass_kernel_spmd

f32 = mybir.dt.float32
bf = mybir.dt.bfloat16
AF = mybir.ActivationFunctionType
ALU = mybir.AluOpType

B, T, C, H = 4, 2048, 2048, 16
THETA = 10000.0
NCORES = 8
DH = C // H              # 128
HPC = H // NCORES        # 2 heads per core
CC = HPC * DH            # 256 channels per core
TOK = B * T              # 8192
NKC = C // 128           # 16 K-chunks
NTC = TOK // 128         # 64 token chunks
SCALE = 1.0 / float(np.sqrt(DH))


DEBUG_DUMP = False
DEBUG_NO_NORM = False


def build():
    nc = bass.Bass()
    # ---- DRAM I/O (per-core tensors; same names on all cores, SPMD) ----
    xt = nc.dram_tensor("xt", [C, TOK], bf, kind="ExternalInput")        # x^T
    wqkv = nc.dram_tensor("wqkv", [C, 3 * CC], bf, kind="ExternalInput")  # per-core cols
    brow = nc.dram_tensor("brow", [1, 3 * CC], bf, kind="ExternalInput")
    cosd = nc.dram_tensor("cosd", [T, CC // 2], f32, kind="ExternalInput")
    sind = nc.dram_tensor("sind", [T, CC // 2], f32, kind="ExternalInput")
    wproj = nc.dram_tensor("wproj", [CC, C], bf, kind="ExternalInput")
    out = nc.dram_tensor("out", [TOK, C], f32, kind="ExternalOutput")
    if DEBUG_DUMP:
        dbg_qT = nc.dram_tensor("dbg_qT", [128, HPC * TOK], bf, kind="ExternalOutput")
        dbg_kT = nc.dram_tensor("dbg_kT", [128, HPC * TOK], bf, kind="ExternalOutput")
        dbg_v = nc.dram_tensor("dbg_v", [128, HPC * NTC * 132], bf, kind="ExternalOutput")
        dbg_e = nc.dram_tensor("dbg_e", [128, 2 * 16 * 512], bf, kind="ExternalOutput")
        dbg_y = nc.dram_tensor("dbg_y", [128, 2 * 16 * 256], bf, kind="ExternalOutput")
        dbg_z = nc.dram_tensor("dbg_z", [128, 8 * 16], f32, kind="ExternalOutput")

    NPART = 128  # rope pair count per core = CC//2

    from contextlib import ExitStack
    es = ExitStack()
    with es:
        # ---- persistent SBUF ----
        w_sb = es.enter_context(nc.sbuf_tensor("w_sb", [128, NKC, 3 * CC], bf))    # 24KB
        b_sb = es.enter_context(nc.sbuf_tensor("b_sb", [1, 3 * CC], bf))
        ones1 = es.enter_context(nc.sbuf_tensor("ones1", [1, 128], bf))
        cos_sb = es.enter_context(nc.sbuf_tensor("cos_sb", [128, 16, NPART], f32))  # 8KB
        sin_sb = es.enter_context(nc.sbuf_tensor("sin_sb", [128, 16, NPART], f32))  # 8KB
        qT_sb = es.enter_context(nc.sbuf_tensor("qT_sb", [128, HPC, TOK], bf))      # 32KB
        kT_sb = es.enter_context(nc.sbuf_tensor("kT_sb", [128, HPC, TOK], bf))      # 32KB
        v_sb = es.enter_context(nc.sbuf_tensor("v_sb", [128, HPC, NTC, 132], bf))   # 33KB
        wp_sb = es.enter_context(nc.sbuf_tensor("wp_sb", [128, HPC, C], bf))        # 8KB
        mbig = es.enter_context(nc.sbuf_tensor("mbig", [128, 896], bf))             # 1.75KB
        ident = es.enter_context(nc.sbuf_tensor("ident", [128, 128], bf))
        # ---- semaphores ----
        sems = {}
        for s in ("s_init", "s_x", "s_gp", "s_pe_qk", "s_pe_v", "s_pe_tr",
                  "s_dve_qk", "s_dve_v", "s_dve_rope", "s_act_tr", "s_pe_s",
                  "s_exp", "s_mask", "s_pe_av", "s_y", "s_pe_yt", "s_act_yt",
                  "s_pe_o", "s_oc", "s_od"):
            sems[s] = es.enter_context(nc.semaphore(s))
        (s_init, s_x, s_gp, s_pe_qk, s_pe_v, s_pe_tr, s_dve_qk, s_dve_v,
         s_dve_rope, s_act_tr, s_pe_s, s_exp, s_mask, s_pe_av, s_y, s_pe_yt,
         s_act_yt, s_pe_o, s_oc, s_od) = (
            sems[k] for k in ("s_init", "s_x", "s_gp", "s_pe_qk", "s_pe_v",
                              "s_pe_tr", "s_dve_qk", "s_dve_v", "s_dve_rope",
                              "s_act_tr", "s_pe_s", "s_exp", "s_mask",
                              "s_pe_av", "s_y", "s_pe_yt", "s_act_yt",
                              "s_pe_o", "s_oc", "s_od"))
        N_INIT = 16 + 1 + 1 + 1 + HPC  # w chunks, brow, cos, sin, wproj rows
        G_SETUP = 4                     # gpsimd setup incs

        # =================== PHASE A: QKV + RoPE + transpose ===================
        with (
            nc.sbuf_tensor("x_sb", [128, 2, NKC, 512], bf) as x_sb,      # 32KB
            nc.sbuf_tensor("qk_st", [128, 2, 512], f32) as qk_st,        # 4KB
            nc.sbuf_tensor("tmp", [128, 4, 128], f32) as tmp,            # 2KB
            nc.sbuf_tensor("q_rp", [128, 2, 256], bf) as q_rp,
            nc.sbuf_tensor("k_rp", [128, 2, 256], bf) as k_rp,
            nc.psum_tensor("ps_qk", [128, 2, 512], f32) as ps_qk,
            nc.psum_tensor("ps_v", [128, 2, 512], f32) as ps_v,
            nc.psum_tensor("ps_tr", [128, 2, 1024], bf) as ps_tr,
            nc.Block() as blk,
        ):
            @blk.sync
            def _(sp):
                for k in range(NKC):
                    sp.dma_start(out=w_sb[:, k, :], in_=wqkv[k * 128:(k + 1) * 128, :]).then_inc(s_init, 16)
                sp.dma_start(out=b_sb[:, :], in_=brow[:, :]).then_inc(s_init, 16)
                # cos/sin: [T, 128] -> [p, tc, j]  (t = 128*tc + p)
                cos_r = cosd.rearrange("(tc p) j -> p tc j", p=128)
                sin_r = sind.rearrange("(tc p) j -> p tc j", p=128)
                sp.dma_start(out=cos_sb[:, :, :], in_=cos_r).then_inc(s_init, 16)
                sp.dma_start(out=sin_sb[:, :, :], in_=sin_r).then_inc(s_init, 16)
                for h in range(HPC):
                    sp.dma_start(out=wp_sb[:, h, :], in_=wproj[h * 128:(h + 1) * 128, :]).then_inc(s_init, 16)
                # x^T groups: group g covers tokens [512g, 512g+512), all 16 k-chunks
                xr = xt.rearrange("(kc p) t -> p kc t", p=128)
                for g in range(16):
                    if g >= 2:
                        sp.wait_ge(s_pe_v, 4 * g - 4)
                    for k in range(NKC):
                        sp.dma_start(out=x_sb[:, g % 2, k, :],
                                     in_=xr[:, k, g * 512:(g + 1) * 512]).then_inc(s_x, 16)

            @blk.gpsimd
            def _(gp):
                make_identity(nc, ident[:, :])
                nc.gpsimd.sem_inc(s_gp, 1)
                gp.memset(ones1[:, :], 1.0).then_inc(s_gp, 1)
                gp.memset(mbig[:, :], 1.0)
                gp.affine_select(out=mbig[:, :], in_=mbig[:, :], compare_op=ALU.is_ge,
                                 fill=0.0, base=-384, channel_multiplier=-1,
                                 pattern=[[1, 896]]).then_inc(s_gp, 1)
                gp.memset(v_sb[:, :, :, 128:129], 1.0).then_inc(s_gp, 1)

            @blk.tensor
            def _(t):
                t.wait_ge(s_init, 16 * N_INIT)
                t.wait_ge(s_gp, G_SETUP)
                tn = 0  # transpose counter
                for c in range(NTC):
                    g, m = c // 4, c % 4
                    rho = c % 2
                    if m == 0:
                        t.wait_ge(s_x, 256 * (g + 1))
                    # q|k chunk: psum[tok128, 512]
                    if c >= 2:
                        t.wait_ge(s_dve_qk, c - 1)
                    t.matmul(ps_qk[:, rho, :], ones1[:, :], b_sb[:, 0:512], start=True, stop=False)
                    for k in range(NKC):
                        mm = t.matmul(ps_qk[:, rho, :], x_sb[:, g % 2, k, m * 128:(m + 1) * 128],
                                      w_sb[:, k, 0:512], start=False, stop=(k == NKC - 1))
                        if k == NKC - 1:
                            mm.then_inc(s_pe_qk, 1)
                    # v chunk: psum[tok128, 256]
                    if c >= 2:
                        t.wait_ge(s_dve_v, c - 1)
                    t.matmul(ps_v[:, rho, 0:256], ones1[:, :], b_sb[:, 512:768], start=True, stop=False)
                    for k in range(NKC):
                        mm = t.matmul(ps_v[:, rho, 0:256], x_sb[:, g % 2, k, m * 128:(m + 1) * 128],
                                      w_sb[:, k, 512:768], start=False, stop=(k == NKC - 1))
                        if k == NKC - 1:
                            mm.then_inc(s_pe_v, 1)
                    # transposes of chunk c-1 (software pipelined)
                    if c >= 1:
                        cp = c - 1
                        t.wait_ge(s_dve_rope, cp + 1)
                        for src, dst in ((q_rp, qT_sb), (k_rp, kT_sb)):
                            for h in range(HPC):
                                if tn >= 2:
                                    t.wait_ge(s_act_tr, tn - 1)
                                t.transpose(ps_tr[:, tn % 2, 0:128],
                                            src[:, cp % 2, h * 128:(h + 1) * 128],
                                            ident[:, :]).then_inc(s_pe_tr, 1)
                                tn += 1
                # final transposes for c = 63
                cp = NTC - 1
                t.wait_ge(s_dve_rope, cp + 1)
                for src, dst in ((q_rp, qT_sb), (k_rp, kT_sb)):
                    for h in range(HPC):
                        t.wait_ge(s_act_tr, tn - 1)
                        t.transpose(ps_tr[:, tn % 2, 0:128],
                                    src[:, cp % 2, h * 128:(h + 1) * 128],
                                    ident[:, :]).then_inc(s_pe_tr, 1)
                        tn += 1

            @blk.scalar
            def _(sc):
                tn = 0
                for c in range(NTC):
                    for src, dst in ((q_rp, qT_sb), (k_rp, kT_sb)):
                        for h in range(HPC):
                            sc.wait_ge(s_pe_tr, tn + 1)
                            sc.copy(dst[:, h, c * 128:(c + 1) * 128],
                                    ps_tr[:, tn % 2, 0:128]).then_inc(s_act_tr, 1)
                            tn += 1

            @blk.vector
            def _(v):
                v.wait_ge(s_init, 16 * N_INIT)
                for c in range(NTC):
                    tc = c % 16
                    rho = c % 2
                    # copy q|k psum -> staging f32
                    v.wait_ge(s_pe_qk, c + 1)
                    v.tensor_copy(qk_st[:, rho, :], ps_qk[:, rho, :]).then_inc(s_dve_qk, 1)
                    # v: psum -> v_sb bf16
                    v.wait_ge(s_pe_v, c + 1)
                    v.tensor_copy(v_sb[:, 0, c, 0:128], ps_v[:, rho, 0:128])
                    v.tensor_copy(v_sb[:, 1, c, 0:128], ps_v[:, rho, 128:256]).then_inc(s_dve_v, 1)
                    # rope on q and k
                    if c >= 2:
                        v.wait_ge(s_pe_tr, 4 * c - 4)
                    C_t = cos_sb[:, tc, :]
                    S_t = sin_sb[:, tc, :]
                    last = None
                    for src_off, dst in ((0, q_rp), (256, k_rp)):
                        X0 = qk_st[:, rho, src_off:src_off + 128]
                        X1 = qk_st[:, rho, src_off + 128:src_off + 256]
                        v.tensor_tensor(tmp[:, 0, :], X0, C_t, ALU.mult)
                        v.tensor_tensor(tmp[:, 1, :], X1, S_t, ALU.mult)
                        v.tensor_tensor(tmp[:, 2, :], X0, S_t, ALU.mult)
                        v.tensor_tensor(tmp[:, 3, :], X1, C_t, ALU.mult)
                        # dst cols = h*128 + x*64 + s; x=0 evens, x=1 odds
                        dv = dst[:, rho, :].rearrange("p (h x s) -> p h x s", h=2, x=2)
                        sv = [tmp[:, i, :].rearrange("p (h s) -> p h s", h=2) for i in range(4)]
                        v.tensor_tensor(dv[:, :, 0, :], sv[0], sv[1], ALU.subtract)
                        last = v.tensor_tensor(dv[:, :, 1, :], sv[2], sv[3], ALU.add)
                    last.then_inc(s_dve_rope, 1)

        # =================== PHASE B: attention + proj ===================
        with (
            nc.sbuf_tensor("z_sb", [128, 8, 16], f32) as z_sb,
            nc.sbuf_tensor("e_sb", [128, 2, 16, 512], bf) as e_sb,       # 32KB
            nc.sbuf_tensor("y_sb", [128, 2, 16, 256], bf) as y_sb,       # 16KB
            nc.sbuf_tensor("yt_st", [128, 2, 128], bf) as yt_st,
            nc.sbuf_tensor("rcp", [128, 1], f32) as rcp,
            nc.sbuf_tensor("zst", [128, 1], f32) as zst,
            nc.sbuf_tensor("o_st", [128, 4, 512], f32) as o_st,          # 8KB
            nc.psum_tensor("ps_s", [128, 2, 512], f32) as ps_s,
            nc.psum_tensor("ps_av", [128, 2, 512], f32) as ps_av,
            nc.psum_tensor("ps_yt", [128, 2, 1024], bf) as ps_yt,
            nc.psum_tensor("ps_o", [128, 2, 512], f32) as ps_o,
            nc.Block() as blk2,
        ):
            @blk2.tensor
            def _(t):
                t.wait_ge(s_dve_v, NTC)
                t.wait_ge(s_act_tr, 4 * NTC)
                blkc = 0   # score-block counter
                avg = 0    # AV group counter
                ytn = 0    # y-transpose counter
                og = 0     # proj psum group counter
                for b in range(B):
                    for h in range(HPC):
                        hb = 2 * b + h
                        for j in range(4):
                            q_sl = qT_sb[:, h, b * 2048 + j * 512: b * 2048 + (j + 1) * 512]
                            for i in range(4 * (j + 1)):
                                if blkc >= 2:
                                    t.wait_ge(s_exp, blkc - 1)
                                t.matmul(ps_s[:, blkc % 2, :],
                                         kT_sb[:, h, b * 2048 + i * 128: b * 2048 + (i + 1) * 128],
                                         q_sl, start=True, stop=True).then_inc(s_pe_s, 1)
                                blkc += 1
                            # AV for this j
                            t.wait_ge(s_exp, blkc)
                            t.wait_ge(s_mask, hb * 16 + 4 * (j + 1))
                            for m in range(4):
                                if avg >= 2:
                                    t.wait_ge(s_y, avg - 1)
                                ni = 4 * j + m + 1
                                for i in range(ni):
                                    mm = t.matmul(ps_av[:, avg % 2, 0:129],
                                                  e_sb[:, hb % 2, i, m * 128:(m + 1) * 128],
                                                  v_sb[:, h, b * 16 + i, 0:129],
                                                  start=(i == 0), stop=(i == ni - 1))
                                    if i == ni - 1:
                                        mm.then_inc(s_pe_av, 1)
                                avg += 1
                    # ---- proj for batch b ----
                    t.wait_ge(s_y, 32 * (b + 1))
                    for m2 in range(16):
                        for h in range(HPC):
                            if ytn >= 2:
                                t.wait_ge(s_act_yt, ytn - 1)
                            t.transpose(ps_yt[:, ytn % 2, 0:128],
                                        y_sb[:, b % 2, m2, h * 128:(h + 1) * 128],
                                        ident[:, :]).then_inc(s_pe_yt, 1)
                            ytn += 1
                        t.wait_ge(s_act_yt, ytn)
                        for n in range(4):
                            if og >= 2:
                                t.wait_ge(s_oc, og - 1)
                            t.matmul(ps_o[:, og % 2, :], yt_st[:, 0, :],
                                     wp_sb[:, 0, n * 512:(n + 1) * 512], start=True, stop=False)
                            t.matmul(ps_o[:, og % 2, :], yt_st[:, 1, :],
                                     wp_sb[:, 1, n * 512:(n + 1) * 512],
                                     start=False, stop=True).then_inc(s_pe_o, 1)
                            og += 1

            @blk2.scalar
            def _(sc):
                blkc = 0
                ytn = 0
                for b in range(B):
                    for h in range(HPC):
                        hb = 2 * b + h
                        for j in range(4):
                            if hb >= 2 and j == 0:
                                sc.wait_ge(s_pe_av, 16 * (hb - 1))
                            if j >= 1:
                                sc.wait_ge(s_pe_av, 16 * hb + 4 * j)
                            for i in range(4 * (j + 1)):
                                sc.wait_ge(s_pe_s, blkc + 1)
                                sc.activation(e_sb[:, hb % 2, i, :], ps_s[:, blkc % 2, :],
                                              AF.Exp, scale=SCALE).then_inc(s_exp, 1)
                                blkc += 1
                    # yT copies for proj(b)
                    for m2 in range(16):
                        gm2 = b * 16 + m2
                        for h in range(HPC):
                            if h == 0 and gm2 >= 1:
                                sc.wait_ge(s_pe_o, 4 * gm2)
                            sc.wait_ge(s_pe_yt, ytn + 1)
                            sc.copy(yt_st[:, h, :], ps_yt[:, ytn % 2, 0:128]).then_inc(s_act_yt, 1)
                            ytn += 1

            @blk2.vector
            def _(v):
                avg = 0
                og = 0
                exp_base = 0
                for b in range(B):
                    for h in range(HPC):
                        hb = 2 * b + h
                        if b >= 2 and h == 0:
                            v.wait_ge(s_pe_o, 64 * (b - 1))
                        for j in range(4):
                            nblk = 4 * (j + 1)
                            # mask the 4 partial blocks i = 4j..4j+3
                            for mp in range(4):
                                i = 4 * j + mp
                                v.wait_ge(s_exp, exp_base + i + 1)
                                e_ap = e_sb[:, hb % 2, i, :]
                                v.tensor_tensor(e_ap, e_ap,
                                                mbig[:, 384 - 128 * mp: 896 - 128 * mp],
                                                ALU.mult).then_inc(s_mask, 1)
                            exp_base += nblk
                            for m in range(4):
                                v.wait_ge(s_pe_av, avg + 1)
                                if DEBUG_NO_NORM:
                                    v.tensor_copy(z_sb[:, hb, 4 * j + m:4 * j + m + 1],
                                                  ps_av[:, avg % 2, 128:129])
                                    v.tensor_copy(y_sb[:, b % 2, 4 * j + m, h * 128:(h + 1) * 128],
                                                  ps_av[:, avg % 2, 0:128]).then_inc(s_y, 1)
                                else:
                                    v.reciprocal(rcp[:, :], ps_av[:, avg % 2, 128:129])
                                    v.drain()
                                    v.tensor_scalar(y_sb[:, b % 2, 4 * j + m, h * 128:(h + 1) * 128],
                                                    ps_av[:, avg % 2, 0:128], rcp[:, :], None,
                                                    ALU.mult).then_inc(s_y, 1)
                                avg += 1
                    # proj copies
                    for m2 in range(16):
                        for n in range(4):
                            v.wait_ge(s_pe_o, og + 1)
                            if og >= 4:
                                v.wait_ge(s_od, 16 * (og - 3))
                            v.tensor_copy(o_st[:, og % 4, :], ps_o[:, og % 2, :]).then_inc(s_oc, 1)
                            og += 1

            @blk2.sync
            def _(sp):
                og = 0
                for b in range(B):
                    for m2 in range(16):
                        c2 = b * 16 + m2
                        for n in range(4):
                            sp.wait_ge(s_oc, og + 1)
                            sp.dma_start(out=out[c2 * 128:(c2 + 1) * 128, n * 512:(n + 1) * 512],
                                         in_=o_st[:, og % 4, :]).then_inc(s_od, 16)
                            og += 1
                sp.wait_ge(s_od, 16 * 256)
                if DEBUG_DUMP:
                    sp.dma_start(out=dbg_qT[:, :], in_=qT_sb[:, :, :].rearrange("p a b -> p (a b)")).then_inc(s_od, 16)
                    sp.dma_start(out=dbg_kT[:, :], in_=kT_sb[:, :, :].rearrange("p a b -> p (a b)")).then_inc(s_od, 16)
                    sp.dma_start(out=dbg_v[:, :], in_=v_sb[:, :, :, :].rearrange("p a b d -> p (a b d)")).then_inc(s_od, 16)
                    sp.dma_start(out=dbg_e[:, :], in_=e_sb[:, :, :, :].rearrange("p a b d -> p (a b d)")).then_inc(s_od, 16)
                    sp.dma_start(out=dbg_y[:, :], in_=y_sb[:, :, :, :].rearrange("p a b d -> p (a b d)")).then_inc(s_od, 16)
                    sp.dma_start(out=dbg_z[:, :], in_=z_sb[:, :, :].rearrange("p a b -> p (a b)")).then_inc(s_od, 16)
                    sp.wait_ge(s_od, 16 * 262)
    return nc


_NC_CACHE = None


def _get_nc():
    global _NC_CACHE
    if _NC_CACHE is None:
        _NC_CACHE = build()
    return _NC_CACHE


def _host_inputs(x, w_qkv, b_qkv, w_proj):
    """Per-core input dicts."""
    xt = np.ascontiguousarray(x.reshape(TOK, C).T).astype(ml_dtypes.bfloat16)
    t = np.arange(T, dtype=np.float64)
    maps = []
    for c in range(NCORES):
        heads = [2 * c, 2 * c + 1]
        # q/k column order: [h0 even, h1 even, h0 odd, h1 odd] (64 each)
        ev = np.concatenate([128 * g + 2 * np.arange(64) for g in heads])
        od = ev + 1
        qk_cols = np.concatenate([ev, od])
        v_cols = np.concatenate([np.arange(128 * g, 128 * g + 128) for g in heads])
        wq = w_qkv[:, 0:C][:, qk_cols]
        wk = w_qkv[:, C:2 * C][:, qk_cols]
        wv = w_qkv[:, 2 * C:3 * C][:, v_cols]
        wc = np.concatenate([wq, wk, wv], axis=1).astype(ml_dtypes.bfloat16)
        bq = b_qkv[0:C][qk_cols]
        bk = b_qkv[C:2 * C][qk_cols]
        bv = b_qkv[2 * C:3 * C][v_cols]
        brow = np.concatenate([bq, bk, bv])[None, :].astype(ml_dtypes.bfloat16)
        # rope tables: pair index j -> global pair P = 128c + j
        P = 128 * c + np.arange(128)
        inv = THETA ** (-(P.astype(np.float64)) / 1024.0)
        ang = t[:, None] * inv[None, :]
        cosd = np.cos(ang).astype(np.float32)
        sind = np.sin(ang).astype(np.float32)
        wp = w_proj[v_cols, :].astype(ml_dtypes.bfloat16)
        maps.append({
            "xt": xt, "wqkv": wc, "brow": brow,
            "cosd": cosd, "sind": sind, "wproj": wp,
        })
    return maps


def kernel(x, w_qkv, b_qkv, w_proj, b_proj):
    x = np.asarray(x, dtype=np.float32)
    w_qkv = np.asarray(w_qkv, dtype=np.float32)
    b_qkv = np.asarray(b_qkv, dtype=np.float32)
    w_proj = np.asarray(w_proj, dtype=np.float32)
    b_proj = np.asarray(b_proj, dtype=np.float32)
    nc = _get_nc()
    in_maps = _host_inputs(x, w_qkv, b_qkv, w_proj)
    res = run_bass_kernel_spmd(nc, in_maps, core_ids=list(range(NCORES)))
    acc = np.zeros((TOK, C), dtype=np.float32)
    for r in res.results:
        acc += r["out"]
    acc += b_proj[None, :]
    return acc.reshape(B, T, C)
